# revision 15
# baseline (speedup 1.0000x reference)
"""Trainium2 Bass kernel for nn_Attn_76424648065726.

Computes softmax(einsum('so,o->s', outputs @ W.T + b, w)) reshaped to
[1, 1, S].

Math: (outputs @ W.T + b) @ w == outputs @ (W.T @ w) + dot(b, w), and the
scalar dot(b, w) cancels inside softmax.  So the kernel computes
softmax(outputs @ v) with v = W.T @ w — turning the [S,H2]x[H2,H2] matmul
into a memory-bound matvec pipeline.

Sharding (8 cores, hidden-dim parallel): core k owns columns
[512k, 512k+512) of both W and outputs (no cross-core data needed until
the energies are summed).
  phase 1: v_k = W[:, cols_k].T @ w            (PE, PSUM accumulate)
  PE-transpose v_k [1,512] -> vt [128,4]
  phase 2: e_k[s] = outputs[s, cols_k] @ v_k   (PE matvec on X^T tiles)
  ReduceScatter(add): core k gets summed energies for s in
    [1024k, 1024k+1024)
  local max/exp/sum, AllGather of the (max, expsum) pairs (8 B/rank),
  rescale own 1024 values, output own chunk; host concatenates.

outputs/W/w are staged to fp16 on the host (halves HBM traffic, faster
PE).  All accumulation is fp32 (PSUM / ACT accumulator).  X is staged
host-side in a transposed, DMA-friendly tiled layout so phase 2 runs on
the otherwise-idle PE instead of DVE+ACT.
"""

import numpy as np

N_CORES = 8
S = 8192
H2 = 4096
HS = H2 // N_CORES  # 512 columns of W / outputs per core
N_OCHUNK = H2 // 128  # 32 contraction chunks for v
WCPG = 4  # o-chunks per W tile (DMA batch)
NHC = HS // 128  # 4 h-chunks per core
NG = 8  # X s-groups per core (1024 s each)
SG = S // NG  # 1024
SC = S // N_CORES  # 1024 output chunk per core

_CACHE = {}


def _build_nc(enable_asserts=False, debug_taps=False):
    import concourse.bass as bass
    import concourse.tile as tile
    from concourse import bacc, mybir

    nc = bacc.Bacc(
        "TRN2",
        target_bir_lowering=False,
        debug=False,
        enable_asserts=enable_asserts,
        num_devices=N_CORES,
    )
    fp32 = mybir.dt.float32
    f16 = mybir.dt.float16
    # x: X[:, cols_k] transposed + grouped:
    #   x[(g*4 + hc)*128 + p, s] = X[1024g + s, 512k + 128hc + p]
    x_d = nc.dram_tensor("x", [NG * HS, SG], f16, kind="ExternalInput").ap()
    wc_d = nc.dram_tensor("wc", [H2, HS], f16, kind="ExternalInput").ap()
    wt_d = nc.dram_tensor("wt", [128, N_OCHUNK], f16, kind="ExternalInput").ap()
    p_d = nc.dram_tensor("p", [128, S // 128], fp32, kind="ExternalOutput").ap()
    dbg = None
    if debug_taps:
        dbg = {
            "dbg_vt": nc.dram_tensor("dbg_vt", [128, NHC], f16,
                                     kind="ExternalOutput").ap(),
            "dbg_e": nc.dram_tensor("dbg_e", [1, S], fp32,
                                    kind="ExternalOutput").ap(),
            "dbg_ef": nc.dram_tensor("dbg_ef", [128, 8], fp32,
                                     kind="ExternalOutput").ap(),
        }

    with tile.TileContext(nc) as tc:
        _body(tc, x_d, wc_d, wt_d, p_d, dbg=dbg)
    nc.compile()
    return nc


def _body(tc, x_d, wc_d, wt_d, p_d, dbg=None):
    import concourse.bass as bass
    from concourse import bass_isa, mybir

    nc = tc.nc
    fp32 = mybir.dt.float32
    f16 = mybir.dt.float16
    ts = bass.ts

    from contextlib import ExitStack

    with ExitStack() as ctx:
        wpool = ctx.enter_context(tc.tile_pool(name="wpool", bufs=8))
        xpool = ctx.enter_context(tc.tile_pool(name="xpool", bufs=8))
        small = ctx.enter_context(tc.tile_pool(name="small", bufs=1))
        dram = ctx.enter_context(tc.tile_pool(name="dram", bufs=1, space="DRAM"))

        # w, pre-transposed on host to [128, 32]: wt[p, c] = w[c*128 + p]
        wt_sb = small.tile([128, N_OCHUNK], f16)
        nc.scalar.dma_start(wt_sb[:], wt_d[:])

        # Dummy 4-byte AllReduce fired immediately (on an uninitialized
        # DRAM scratch tile — the result is ignored, so no input dep):
        # absorbs the cross-core launch/CC-stream-init skew concurrently
        # with the DMA stream, so the real energy AllReduce later finds
        # the ranks aligned and the CC stream warm.
        w0_dr = dram.tile([1, 1], fp32)
        w0_out = dram.tile([1, 1], fp32)
        nc.gpsimd.collective_compute(
            "AllReduce",
            mybir.AluOpType.add,
            replica_groups=[list(range(N_CORES))],
            ins=[w0_dr.opt()],
            outs=[w0_out.opt()],
        )

        # constants for the PE-based partition reduce/broadcast in the
        # softmax tail; built on idle engines during the DMA shadow.
        from concourse import masks

        id128 = small.tile([128, 128], fp32)
        masks.make_identity(nc, id128[:])
        ones_r = small.tile([1, 128], fp32)
        nc.vector.memset(ones_r[:], 1.0)
        ones_c = small.tile([128, 1], fp32)
        nc.vector.memset(ones_c[:], 1.0)

        # All streaming DMAs go on the sync HWDGE ring in FIFO order: W
        # first (it gates phase 1), then X.  One ring keeps HBM busy;
        # both rings share the same 16 SDMA engines so spreading gains
        # nothing.  ~0.5-1MiB slices amortize ring overhead.
        wtiles = []
        for g in range(N_OCHUNK // WCPG):
            wtile = wpool.tile([128, WCPG, HS], f16)
            nc.sync.dma_start(
                wtile[:],
                wc_d[ts(g, 128 * WCPG), :].rearrange("(c p) j -> p c j", p=128),
            )
            wtiles.append(wtile)

        xtiles = []
        for g in range(NG):
            xt = xpool.tile([128, NHC, SG], f16)
            nc.sync.dma_start(
                xt[:],
                x_d[ts(g, NHC * 128), :].rearrange("(h p) s -> p h s", p=128),
            )
            xtiles.append(xt)

        with ExitStack() as p1ctx:
            # PE warmup: the HAM throttles a cold PE to 1.2 GHz; ~10us of
            # dummy matmuls on memset data while W streams in gets the real
            # matmuls the 2.4 GHz rate.
            wu_pool = p1ctx.enter_context(tc.tile_pool(name="wu_pool", bufs=1))
            wu_psum = p1ctx.enter_context(
                tc.tile_pool(name="wu_psum", bufs=1, space="PSUM")
            )
            vpsum = p1ctx.enter_context(
                tc.tile_pool(name="vpsum", bufs=1, space="PSUM")
            )
            vtpsum = p1ctx.enter_context(
                tc.tile_pool(name="vtpsum", bufs=1, space="PSUM")
            )
            wu_lhs = wu_pool.tile([128, 1], f16)
            wu_rhs = wu_pool.tile([128, HS], f16)
            nc.vector.memset(wu_lhs[:], 0.0)
            nc.vector.memset(wu_rhs[:], 0.0)
            wu_ps = wu_psum.tile([1, HS], fp32)
            for i in range(10):
                nc.tensor.matmul(
                    wu_ps[:], lhsT=wu_lhs[:], rhs=wu_rhs[:], start=True, stop=True
                )
            # short (N=128) dummies bridge the gap until W arrives — a ~2us
            # PE idle re-throttles the HAM.
            for i in range(20):
                nc.tensor.matmul(
                    wu_ps[:, :128], lhsT=wu_lhs[:], rhs=wu_rhs[:, :128],
                    start=True, stop=True,
                )

            # ---- phase 1: v = W_k.T @ w  ([1, HS] accumulated in PSUM) ----
            v_ps = vpsum.tile([1, HS], fp32)
            for c in range(N_OCHUNK):
                nc.tensor.matmul(
                    v_ps[:],
                    lhsT=wt_sb[:, c : c + 1],
                    rhs=wtiles[c // WCPG][:, c % WCPG, :],
                    start=(c == 0),
                    stop=(c == N_OCHUNK - 1),
                )

            v_row = small.tile([1, HS], fp32)
            nc.vector.tensor_copy(v_row[:], v_ps[:])

            # PE-transpose v [1, 512] -> vt [128, 4] (vt[p, hc] = v[128hc+p])
            # fp32 keeps the PSUM column slices 4-byte aligned.
            id1 = wu_pool.tile([1, 1], fp32)
            nc.vector.memset(id1[:], 1.0)
            vt_ps = vtpsum.tile([128, NHC], fp32)
            for hc in range(NHC):
                nc.tensor.transpose(
                    vt_ps[:, hc : hc + 1], v_row[:, ts(hc, 128)], id1[:]
                )
            vt = small.tile([128, NHC], f16)
            nc.vector.tensor_copy(vt[:], vt_ps[:])
            if dbg is not None:
                nc.scalar.dma_start(dbg["dbg_vt"][:], vt[:])

        # ---- phase 2: partial energies e[s] = X[s, cols_k] @ v_k on PE ----
        # e lives as [1, 8192] fp32 on partition 0.
        e_sb = small.tile([1, S], fp32)
        with ExitStack() as p2ctx:
            epool = p2ctx.enter_context(
                tc.tile_pool(name="epsum", bufs=8, space="PSUM")
            )
            for g in range(NG):
                for half in range(2):
                    e_ps = epool.tile([1, 512], fp32)
                    for hc in range(NHC):
                        nc.tensor.matmul(
                            e_ps[:],
                            lhsT=vt[:, hc : hc + 1],
                            rhs=xtiles[g][:, hc, ts(half, 512)],
                            start=(hc == 0),
                            stop=(hc == NHC - 1),
                        )
                    nc.vector.tensor_copy(
                        e_sb[:, g * SG + half * 512 : g * SG + half * 512 + 512],
                        e_ps[:],
                    )

        # ---- single AllReduce of the 32 KiB energy vector ----
        e_dr = dram.tile([1, S], fp32)
        e_red = dram.tile([1, S], fp32)
        nc.scalar.dma_start(e_dr[:], e_sb[:])
        nc.gpsimd.collective_compute(
            "AllReduce",
            mybir.AluOpType.add,
            replica_groups=[list(range(N_CORES))],
            ins=[e_dr.opt()],
            outs=[e_red.opt()],
        )
        # land the summed energies partition-spread: ef128[p, c] = e[64p + c]
        ef128 = small.tile([128, S // 128], fp32)
        nc.scalar.dma_start(
            ef128[:], e_red[:].rearrange("o (p c) -> (o p) c", p=128)
        )
        if dbg is not None:
            nc.scalar.dma_start(dbg["dbg_e"][:], e_sb[:])
            nc.scalar.dma_start(dbg["dbg_ef"][:], ef128[:, :8])

        # ---- softmax over all S values (redundant on every core) ----
        # partition reduces / broadcasts run on the idle PE via matmuls.
        with ExitStack() as tctx:
            tpsum = tctx.enter_context(
                tc.tile_pool(name="tpsum", bufs=1, space="PSUM")
            )
            m1 = small.tile([128, 1], fp32)
            nc.vector.tensor_reduce(
                m1[:], ef128[:], axis=mybir.AxisListType.X, op=mybir.AluOpType.max
            )
            mT_ps = tpsum.tile([1, 128], fp32)
            nc.tensor.transpose(mT_ps[:], m1[:], id128[:])
            mT = small.tile([1, 128], fp32)
            nc.vector.tensor_copy(mT[:], mT_ps[:])
            M = small.tile([1, 1], fp32)
            nc.vector.tensor_reduce(
                M[:], mT[:], axis=mybir.AxisListType.X, op=mybir.AluOpType.max
            )
            nM = small.tile([1, 1], fp32)
            nc.scalar.mul(nM[:], M[:], -1.0)
            nmb_ps = tpsum.tile([128, 1], fp32)
            nc.tensor.matmul(
                nmb_ps[:], lhsT=ones_r[:], rhs=nM[:], start=True, stop=True
            )
            nmb = small.tile([128, 1], fp32)
            nc.vector.tensor_copy(nmb[:], nmb_ps[:])

            pexp = small.tile([128, S // 128], fp32)
            s1 = small.tile([128, 1], fp32)
            nc.scalar.activation(
                pexp[:],
                ef128[:],
                mybir.ActivationFunctionType.Exp,
                bias=nmb[:],
                scale=1.0,
                accum_out=s1[:],
            )
            z_ps = tpsum.tile([1, 1], fp32)
            nc.tensor.matmul(
                z_ps[:], lhsT=s1[:], rhs=ones_c[:], start=True, stop=True
            )
            z = small.tile([1, 1], fp32)
            nc.vector.tensor_copy(z[:], z_ps[:])
            rz = small.tile([1, 1], fp32)
            nc.vector.reciprocal(rz[:], z[:])
            rzb_ps = tpsum.tile([128, 1], fp32)
            nc.tensor.matmul(
                rzb_ps[:], lhsT=ones_r[:], rhs=rz[:], start=True, stop=True
            )
            rzb = small.tile([128, 1], fp32)
            nc.vector.tensor_copy(rzb[:], rzb_ps[:])

            po = small.tile([128, S // 128], fp32)
            nc.scalar.mul(po[:], pexp[:], rzb[:])
            nc.scalar.dma_start(p_d[:], po[:])


def _shard_inputs(outputs, W, w):
    f16 = np.float16
    outputs = np.asarray(outputs, dtype=np.float32)
    W = np.asarray(W, dtype=np.float32)
    w = np.asarray(w, dtype=np.float32)
    wt = np.ascontiguousarray(w.reshape(N_OCHUNK, 128).T).astype(f16)
    in_maps = []
    for k in range(N_CORES):
        cols = slice(HS * k, HS * (k + 1))
        xk = outputs[:, cols]  # [8192, 512]
        # [(g, hc*128+p), s] = X[1024g + s, 512k + 128hc + p]
        xt = (
            np.ascontiguousarray(xk.reshape(NG, SG, HS).transpose(0, 2, 1))
            .reshape(NG * HS, SG)
            .astype(f16)
        )
        in_maps.append(
            {
                "x": xt,
                "wc": np.ascontiguousarray(W[:, cols]).astype(f16),
                "wt": wt,
            }
        )
    return in_maps


def _run(outputs, W, w, trace=False, trace_cores=None):
    from concourse.bass_utils import run_bass_kernel_spmd

    if "nc" not in _CACHE:
        _CACHE["nc"] = _build_nc()
    nc = _CACHE["nc"]
    in_maps = _shard_inputs(outputs, W, w)
    res = run_bass_kernel_spmd(
        nc, in_maps, list(range(N_CORES)), trace=trace, trace_cores=trace_cores
    )
    # p[p, c] = softmax(e)[64p + c]; row-major reshape restores s-order
    full = np.asarray(res.results[0]["p"]).reshape(1, 1, S).astype(np.float32)
    return full, res


def kernel(outputs, W, b, w):
    out, _ = _run(outputs, W, w, trace=False)
    return out


def kernel_traced(outputs, W, b, w, trace_cores=None):
    out, res = _run(outputs, W, w, trace=True, trace_cores=trace_cores)
    return out, res


# revision 16
# speedup vs baseline: 1.0280x; 1.0280x over previous
"""Trainium2 Bass kernel for nn_Attn_76424648065726.

Computes softmax(einsum('so,o->s', outputs @ W.T + b, w)) reshaped to
[1, 1, S].

Math: (outputs @ W.T + b) @ w == outputs @ (W.T @ w) + dot(b, w), and the
scalar dot(b, w) cancels inside softmax.  So the kernel computes
softmax(outputs @ v) with v = W.T @ w — turning the [S,H2]x[H2,H2] matmul
into a memory-bound matvec pipeline.

Sharding (8 cores, hidden-dim parallel): core k owns columns
[512k, 512k+512) of both W and outputs (no cross-core data needed until
the energies are summed).
  phase 1: v_k = W[:, cols_k].T @ w            (PE, PSUM accumulate)
  PE-transpose v_k [1,512] -> vt [128,4]
  phase 2: e_k[s] = outputs[s, cols_k] @ v_k   (PE matvec on X^T tiles)
  ReduceScatter(add): core k gets summed energies for s in
    [1024k, 1024k+1024)
  local max/exp/sum, AllGather of the (max, expsum) pairs (8 B/rank),
  rescale own 1024 values, output own chunk; host concatenates.

outputs/W/w are staged to fp16 on the host (halves HBM traffic, faster
PE).  All accumulation is fp32 (PSUM / ACT accumulator).  X is staged
host-side in a transposed, DMA-friendly tiled layout so phase 2 runs on
the otherwise-idle PE instead of DVE+ACT.
"""

import numpy as np

N_CORES = 8
S = 8192
H2 = 4096
HS = H2 // N_CORES  # 512 columns of W / outputs per core
N_OCHUNK = H2 // 128  # 32 contraction chunks for v
WCPG = 4  # o-chunks per W tile (DMA batch)
NHC = HS // 128  # 4 h-chunks per core
NG = 8  # X s-groups per core (1024 s each)
SG = S // NG  # 1024
SC = S // N_CORES  # 1024 output chunk per core

_CACHE = {}


def _build_nc(enable_asserts=False, debug_taps=False):
    import concourse.bass as bass
    import concourse.tile as tile
    from concourse import bacc, mybir

    nc = bacc.Bacc(
        "TRN2",
        target_bir_lowering=False,
        debug=False,
        enable_asserts=enable_asserts,
        num_devices=N_CORES,
    )
    fp32 = mybir.dt.float32
    f16 = mybir.dt.float16
    # x: X[:, cols_k] transposed + grouped:
    #   x[(g*4 + hc)*128 + p, s] = X[1024g + s, 512k + 128hc + p]
    x_d = nc.dram_tensor("x", [NG * HS, SG], f16, kind="ExternalInput").ap()
    wc_d = nc.dram_tensor("wc", [H2, HS], f16, kind="ExternalInput").ap()
    wt_d = nc.dram_tensor("wt", [128, N_OCHUNK], f16, kind="ExternalInput").ap()
    p_d = nc.dram_tensor("p", [128, S // 128], fp32, kind="ExternalOutput").ap()
    dbg = None
    if debug_taps:
        dbg = {
            "dbg_vt": nc.dram_tensor("dbg_vt", [128, NHC], f16,
                                     kind="ExternalOutput").ap(),
            "dbg_e": nc.dram_tensor("dbg_e", [1, S], fp32,
                                    kind="ExternalOutput").ap(),
            "dbg_ef": nc.dram_tensor("dbg_ef", [128, 8], fp32,
                                     kind="ExternalOutput").ap(),
        }

    with tile.TileContext(nc) as tc:
        _body(tc, x_d, wc_d, wt_d, p_d, dbg=dbg)
    nc.compile()
    return nc


def _body(tc, x_d, wc_d, wt_d, p_d, dbg=None):
    import concourse.bass as bass
    from concourse import bass_isa, mybir

    nc = tc.nc
    fp32 = mybir.dt.float32
    f16 = mybir.dt.float16
    ts = bass.ts

    from contextlib import ExitStack

    with ExitStack() as ctx:
        wpool = ctx.enter_context(tc.tile_pool(name="wpool", bufs=8))
        xpool = ctx.enter_context(tc.tile_pool(name="xpool", bufs=8))
        small = ctx.enter_context(tc.tile_pool(name="small", bufs=1))
        dram = ctx.enter_context(tc.tile_pool(name="dram", bufs=1, space="DRAM"))

        # w, pre-transposed on host to [128, 32]: wt[p, c] = w[c*128 + p]
        wt_sb = small.tile([128, N_OCHUNK], f16)
        nc.scalar.dma_start(wt_sb[:], wt_d[:])

        # Dummy 4-byte AllReduce fired immediately (on an uninitialized
        # DRAM scratch tile — the result is ignored, so no input dep):
        # absorbs the cross-core launch/CC-stream-init skew concurrently
        # with the DMA stream, so the real energy AllReduce later finds
        # the ranks aligned and the CC stream warm.
        w0_dr = dram.tile([1, 1], fp32)
        w0_out = dram.tile([1, 1], fp32)
        nc.gpsimd.collective_compute(
            "AllReduce",
            mybir.AluOpType.add,
            replica_groups=[list(range(N_CORES))],
            ins=[w0_dr.opt()],
            outs=[w0_out.opt()],
        )
        w1_dr = dram.tile([1, 1], fp32)
        w1_out = dram.tile([1, 1], fp32)
        nc.gpsimd.collective_compute(
            "AllReduce",
            mybir.AluOpType.add,
            replica_groups=[list(range(N_CORES))],
            ins=[w1_dr.opt()],
            outs=[w1_out.opt()],
        )

        # constants for the PE-based partition reduce/broadcast in the
        # softmax tail; built on idle engines during the DMA shadow.
        from concourse import masks

        id128 = small.tile([128, 128], fp32)
        masks.make_identity(nc, id128[:])
        ones_r = small.tile([1, 128], fp32)
        nc.vector.memset(ones_r[:], 1.0)
        ones_c = small.tile([128, 1], fp32)
        nc.vector.memset(ones_c[:], 1.0)

        # All streaming DMAs go on the sync HWDGE ring in FIFO order: W
        # first (it gates phase 1), then X.  One ring keeps HBM busy;
        # both rings share the same 16 SDMA engines so spreading gains
        # nothing.  ~0.5-1MiB slices amortize ring overhead.
        wtiles = []
        for g in range(N_OCHUNK // WCPG):
            wtile = wpool.tile([128, WCPG, HS], f16)
            nc.sync.dma_start(
                wtile[:],
                wc_d[ts(g, 128 * WCPG), :].rearrange("(c p) j -> p c j", p=128),
            )
            wtiles.append(wtile)

        xtiles = []
        for g in range(NG):
            xt = xpool.tile([128, NHC, SG], f16)
            nc.sync.dma_start(
                xt[:],
                x_d[ts(g, NHC * 128), :].rearrange("(h p) s -> p h s", p=128),
            )
            xtiles.append(xt)

        with ExitStack() as p1ctx:
            # PE warmup: the HAM throttles a cold PE to 1.2 GHz; ~10us of
            # dummy matmuls on memset data while W streams in gets the real
            # matmuls the 2.4 GHz rate.
            wu_pool = p1ctx.enter_context(tc.tile_pool(name="wu_pool", bufs=1))
            wu_psum = p1ctx.enter_context(
                tc.tile_pool(name="wu_psum", bufs=1, space="PSUM")
            )
            vpsum = p1ctx.enter_context(
                tc.tile_pool(name="vpsum", bufs=1, space="PSUM")
            )
            vtpsum = p1ctx.enter_context(
                tc.tile_pool(name="vtpsum", bufs=1, space="PSUM")
            )
            wu_lhs = wu_pool.tile([128, 1], f16)
            wu_rhs = wu_pool.tile([128, HS], f16)
            nc.vector.memset(wu_lhs[:], 0.0)
            nc.vector.memset(wu_rhs[:], 0.0)
            wu_ps = wu_psum.tile([1, HS], fp32)
            for i in range(10):
                nc.tensor.matmul(
                    wu_ps[:], lhsT=wu_lhs[:], rhs=wu_rhs[:], start=True, stop=True
                )
            # short (N=128) dummies bridge the gap until W arrives — a ~2us
            # PE idle re-throttles the HAM.
            for i in range(20):
                nc.tensor.matmul(
                    wu_ps[:, :128], lhsT=wu_lhs[:], rhs=wu_rhs[:, :128],
                    start=True, stop=True,
                )

            # ---- phase 1: v = W_k.T @ w  ([1, HS] accumulated in PSUM) ----
            v_ps = vpsum.tile([1, HS], fp32)
            for c in range(N_OCHUNK):
                nc.tensor.matmul(
                    v_ps[:],
                    lhsT=wt_sb[:, c : c + 1],
                    rhs=wtiles[c // WCPG][:, c % WCPG, :],
                    start=(c == 0),
                    stop=(c == N_OCHUNK - 1),
                )

            v_row = small.tile([1, HS], fp32)
            nc.vector.tensor_copy(v_row[:], v_ps[:])

            # PE-transpose v [1, 512] -> vt [128, 4] (vt[p, hc] = v[128hc+p])
            # fp32 keeps the PSUM column slices 4-byte aligned.
            id1 = wu_pool.tile([1, 1], fp32)
            nc.vector.memset(id1[:], 1.0)
            vt_ps = vtpsum.tile([128, NHC], fp32)
            for hc in range(NHC):
                nc.tensor.transpose(
                    vt_ps[:, hc : hc + 1], v_row[:, ts(hc, 128)], id1[:]
                )
            vt = small.tile([128, NHC], f16)
            nc.vector.tensor_copy(vt[:], vt_ps[:])
            if dbg is not None:
                nc.scalar.dma_start(dbg["dbg_vt"][:], vt[:])

        # ---- phase 2: partial energies e[s] = X[s, cols_k] @ v_k on PE ----
        # e lives as [1, 8192] fp32 on partition 0.
        e_sb = small.tile([1, S], fp32)
        with ExitStack() as p2ctx:
            epool = p2ctx.enter_context(
                tc.tile_pool(name="epsum", bufs=8, space="PSUM")
            )
            for g in range(NG):
                for half in range(2):
                    e_ps = epool.tile([1, 512], fp32)
                    for hc in range(NHC):
                        nc.tensor.matmul(
                            e_ps[:],
                            lhsT=vt[:, hc : hc + 1],
                            rhs=xtiles[g][:, hc, ts(half, 512)],
                            start=(hc == 0),
                            stop=(hc == NHC - 1),
                        )
                    nc.vector.tensor_copy(
                        e_sb[:, g * SG + half * 512 : g * SG + half * 512 + 512],
                        e_ps[:],
                    )

        # ---- single AllReduce of the 32 KiB energy vector ----
        e_dr = dram.tile([1, S], fp32)
        e_red = dram.tile([1, S], fp32)
        nc.scalar.dma_start(e_dr[:], e_sb[:])
        nc.gpsimd.collective_compute(
            "AllReduce",
            mybir.AluOpType.add,
            replica_groups=[list(range(N_CORES))],
            ins=[e_dr.opt()],
            outs=[e_red.opt()],
        )
        # land the summed energies partition-spread: ef128[p, c] = e[64p + c]
        ef128 = small.tile([128, S // 128], fp32)
        nc.scalar.dma_start(
            ef128[:], e_red[:].rearrange("o (p c) -> (o p) c", p=128)
        )
        if dbg is not None:
            nc.scalar.dma_start(dbg["dbg_e"][:], e_sb[:])
            nc.scalar.dma_start(dbg["dbg_ef"][:], ef128[:, :8])

        # ---- softmax over all S values (redundant on every core) ----
        # partition reduces / broadcasts run on the idle PE via matmuls.
        with ExitStack() as tctx:
            tpsum = tctx.enter_context(
                tc.tile_pool(name="tpsum", bufs=1, space="PSUM")
            )
            m1 = small.tile([128, 1], fp32)
            nc.vector.tensor_reduce(
                m1[:], ef128[:], axis=mybir.AxisListType.X, op=mybir.AluOpType.max
            )
            mT_ps = tpsum.tile([1, 128], fp32)
            nc.tensor.transpose(mT_ps[:], m1[:], id128[:])
            mT = small.tile([1, 128], fp32)
            nc.vector.tensor_copy(mT[:], mT_ps[:])
            M = small.tile([1, 1], fp32)
            nc.vector.tensor_reduce(
                M[:], mT[:], axis=mybir.AxisListType.X, op=mybir.AluOpType.max
            )
            nM = small.tile([1, 1], fp32)
            nc.scalar.mul(nM[:], M[:], -1.0)
            nmb_ps = tpsum.tile([128, 1], fp32)
            nc.tensor.matmul(
                nmb_ps[:], lhsT=ones_r[:], rhs=nM[:], start=True, stop=True
            )
            nmb = small.tile([128, 1], fp32)
            nc.vector.tensor_copy(nmb[:], nmb_ps[:])

            pexp = small.tile([128, S // 128], fp32)
            s1 = small.tile([128, 1], fp32)
            nc.scalar.activation(
                pexp[:],
                ef128[:],
                mybir.ActivationFunctionType.Exp,
                bias=nmb[:],
                scale=1.0,
                accum_out=s1[:],
            )
            z_ps = tpsum.tile([1, 1], fp32)
            nc.tensor.matmul(
                z_ps[:], lhsT=s1[:], rhs=ones_c[:], start=True, stop=True
            )
            z = small.tile([1, 1], fp32)
            nc.vector.tensor_copy(z[:], z_ps[:])
            rz = small.tile([1, 1], fp32)
            nc.vector.reciprocal(rz[:], z[:])
            rzb_ps = tpsum.tile([128, 1], fp32)
            nc.tensor.matmul(
                rzb_ps[:], lhsT=ones_r[:], rhs=rz[:], start=True, stop=True
            )
            rzb = small.tile([128, 1], fp32)
            nc.vector.tensor_copy(rzb[:], rzb_ps[:])

            po = small.tile([128, S // 128], fp32)
            nc.scalar.mul(po[:], pexp[:], rzb[:])
            nc.scalar.dma_start(p_d[:], po[:])


def _shard_inputs(outputs, W, w):
    f16 = np.float16
    outputs = np.asarray(outputs, dtype=np.float32)
    W = np.asarray(W, dtype=np.float32)
    w = np.asarray(w, dtype=np.float32)
    wt = np.ascontiguousarray(w.reshape(N_OCHUNK, 128).T).astype(f16)
    in_maps = []
    for k in range(N_CORES):
        cols = slice(HS * k, HS * (k + 1))
        xk = outputs[:, cols]  # [8192, 512]
        # [(g, hc*128+p), s] = X[1024g + s, 512k + 128hc + p]
        xt = (
            np.ascontiguousarray(xk.reshape(NG, SG, HS).transpose(0, 2, 1))
            .reshape(NG * HS, SG)
            .astype(f16)
        )
        in_maps.append(
            {
                "x": xt,
                "wc": np.ascontiguousarray(W[:, cols]).astype(f16),
                "wt": wt,
            }
        )
    return in_maps


def _run(outputs, W, w, trace=False, trace_cores=None):
    from concourse.bass_utils import run_bass_kernel_spmd

    if "nc" not in _CACHE:
        _CACHE["nc"] = _build_nc()
    nc = _CACHE["nc"]
    in_maps = _shard_inputs(outputs, W, w)
    res = run_bass_kernel_spmd(
        nc, in_maps, list(range(N_CORES)), trace=trace, trace_cores=trace_cores
    )
    # p[p, c] = softmax(e)[64p + c]; row-major reshape restores s-order
    full = np.asarray(res.results[0]["p"]).reshape(1, 1, S).astype(np.float32)
    return full, res


def kernel(outputs, W, b, w):
    out, _ = _run(outputs, W, w, trace=False)
    return out


def kernel_traced(outputs, W, b, w, trace_cores=None):
    out, res = _run(outputs, W, w, trace=True, trace_cores=trace_cores)
    return out, res
